# revision 17
# baseline (speedup 1.0000x reference)
"""4D multilinear interpolation (8x8x8x8 lattice) on 8 Trainium2 cores.

For each row b: scale coordinates[b] (4 values in [0,1)) to the 7-cell
lattice, find the containing cell, gather the 16 corner values from
mesh_pred[b] (4096 values), and blend with multilinear weights.

HW constraint (measured): indirect DMA gather consumes ONE index per
partition and streams the dest free-width contiguously from it (multi-
index offset APs abort at runtime, with or without bounds_check).  So
rows are laid out b = n*128 + p (host pre-permutes coordinates into
(p,n) order; output is permuted back) and each of the 32 gathers
fetches, per partition, the 586-float span that covers all 16 cell
corners of one row.  The gather pitch is ~1.4us/instruction
(SWDGE-paced), so the kernel is structured to hide everything else
under it: the index chain runs in two stages (first 4 row-tiles, then
the rest) so gather 0 issues early, the weight build and group-wise
blends overlap later gathers, and outputs stream out per group.
"""

import numpy as np

import concourse.bass as bass
import concourse.bacc as bacc
import concourse.mybir as mybir
from concourse import bass_utils
from concourse.tile import TileContext

P = 128          # partitions
I = 32           # row-tiles (gathers) per core
GB = 8           # row-tiles per blend group
BC = P * I       # 4096 rows per core
VOL = 4096       # 8^4 lattice values per row
ND = 4
NCORES = 8
MESH = 8
SPANW = 640      # padded per-row gather width (586 used)
SPAN = 586       # 585 max corner offset + 1
F32 = mybir.dt.float32
I32 = mybir.dt.int32
OP = mybir.AluOpType


def _build():
    nc = bacc.Bacc("TRN2", target_bir_lowering=False, debug=False,
                   dynamic_dma_scratch_size=65536)
    # coordinates arrive host-permuted: device row p*I+n = original row n*P+p
    coords = nc.dram_tensor("coordinates", [BC, ND], F32, kind="ExternalInput")
    mesh = nc.dram_tensor("mesh_pred", [BC, VOL], F32, kind="ExternalInput")
    # host-precomputed (n*P+p)*8 as f32, laid out [p, n] (exact below 2^24)
    rowf8 = nc.dram_tensor("rowf8", [P, I], F32, kind="ExternalInput")
    out = nc.dram_tensor("out", [BC], F32, kind="ExternalOutput")

    mesh_2d = mesh[:]
    coords_t = coords[:].rearrange("(p n) d -> p (n d)", p=P)
    out_t = out[:].rearrange("(p n) -> p n", p=P)  # host permutes back

    with TileContext(nc) as tc:
        with tc.tile_pool(name="pool", bufs=1) as pool:
            ct = pool.tile([P, I * ND], F32, tag="ct")
            nc.sync.dma_start(out=ct[:], in_=coords_t)
            rbase = pool.tile([P, I], F32, tag="rbase")
            nc.scalar.dma_start(out=rbase[:], in_=rowf8[:])

            c = pool.tile([P, I * ND], F32, tag="c")
            frac = pool.tile([P, I * ND], F32, tag="frac")
            cif = pool.tile([P, I * ND], F32, tag="cif")
            ci_r = pool.tile([P, I * ND], I32, tag="ci_r")
            ci_f = pool.tile([P, I * ND], F32, tag="ci_f")
            gtt = pool.tile([P, I * ND], F32, tag="gtt")
            t0 = pool.tile([P, I], F32, tag="t0")
            t1 = pool.tile([P, I], F32, tag="t1")
            t2 = pool.tile([P, I], F32, tag="t2")
            t3 = pool.tile([P, I], F32, tag="t3")
            t4 = pool.tile([P, I], F32, tag="t4")
            t5 = pool.tile([P, I], F32, tag="t5")
            t6 = pool.tile([P, I], F32, tag="t6")
            idx = pool.tile([P, I], I32, tag="idx")

            # idx = (((row*8 + d0)*8 + d1)*8 + d2)*8 + d3, all exact in f32.
            # single full-width stage: a split early stage lets the scheduler
            # interleave the wide stage into the serial chain, which delays
            # the first gather instead of advancing it (measured).
            for lo, hi in ((0, I),):
                cs = slice(lo * ND, hi * ND)
                ns = slice(lo, hi)
                nc.vector.tensor_scalar_mul(c[:, cs], ct[:, cs], float(MESH - 1))
                # floor(c) via round-trip cast + fixup (works for any cast
                # rounding mode: the cast lands on floor or floor+1, is_gt
                # subtracts the overshoot); frac = c - floor(c)
                nc.vector.tensor_copy(out=ci_r[:, cs], in_=c[:, cs])
                nc.vector.tensor_copy(out=ci_f[:, cs], in_=ci_r[:, cs])
                nc.vector.tensor_tensor(
                    out=gtt[:, cs], in0=ci_f[:, cs], in1=c[:, cs], op=OP.is_gt
                )
                nc.vector.tensor_tensor(
                    out=cif[:, cs], in0=ci_f[:, cs], in1=gtt[:, cs], op=OP.subtract
                )
                nc.vector.tensor_tensor(
                    out=frac[:, cs], in0=c[:, cs], in1=cif[:, cs], op=OP.subtract
                )
                d = [cif[:, lo * ND + k::ND] for k in range(ND)]
                # slice columns [lo:hi) of the strided view
                d = [bass.AP(v.tensor, v.offset, [v.ap[0], [ND, hi - lo]])
                     for v in d]
                nc.vector.tensor_tensor(
                    out=t0[:, ns], in0=rbase[:, ns], in1=d[0], op=OP.add)
                nc.vector.tensor_scalar_mul(t1[:, ns], t0[:, ns], 8.0)
                nc.vector.tensor_tensor(
                    out=t2[:, ns], in0=t1[:, ns], in1=d[1], op=OP.add)
                nc.vector.tensor_scalar_mul(t3[:, ns], t2[:, ns], 8.0)
                nc.vector.tensor_tensor(
                    out=t4[:, ns], in0=t3[:, ns], in1=d[2], op=OP.add)
                nc.vector.tensor_scalar_mul(t5[:, ns], t4[:, ns], 8.0)
                nc.vector.tensor_tensor(
                    out=t6[:, ns], in0=t5[:, ns], in1=d[3], op=OP.add)
                nc.vector.tensor_copy(out=idx[:, ns], in_=t6[:, ns])

            Gbig = pool.tile([P, I * SPANW], F32, tag="Gbig")
            for n in range(0, I):
                nc.gpsimd.indirect_dma_start(
                    out=Gbig[:, n * SPANW:n * SPANW + SPAN],
                    out_offset=None,
                    in_=mesh_2d,
                    in_offset=bass.IndirectOffsetOnAxis(
                        ap=idx[:, n:n + 1], axis=1),
                    element_offset=0,
                )

            # weights: om=1-frac; w01[(g,n)], w23[(j,n)]; W16[(n,k)] k=(a,b,c,d)
            om = pool.tile([P, I * ND], F32, tag="om")
            nc.vector.tensor_scalar(
                out=om[:], in0=frac[:], scalar1=-1.0, scalar2=1.0,
                op0=OP.mult, op1=OP.add,
            )
            w01 = pool.tile([P, 4 * I], F32, tag="w01")
            w23 = pool.tile([P, 4 * I], F32, tag="w23")
            pairs = ((0, 0), (0, 1), (1, 0), (1, 1))
            for g, (a, b) in enumerate(pairs):
                nc.vector.tensor_tensor(
                    out=w23[:, g * I:(g + 1) * I],
                    in0=(frac if a else om)[:, 2::ND],
                    in1=(frac if b else om)[:, 3::ND], op=OP.mult,
                )
            for g, (a, b) in enumerate(pairs):
                nc.vector.tensor_tensor(
                    out=w01[:, g * I:(g + 1) * I],
                    in0=(frac if a else om)[:, 0::ND],
                    in1=(frac if b else om)[:, 1::ND], op=OP.mult,
                )
            W16 = pool.tile([P, I * 16], F32, tag="W16")  # layout (n, k) k fastest
            for k in range(16):
                g, j = k >> 2, k & 3
                nc.vector.tensor_tensor(
                    out=W16[:, k::16],
                    in0=w01[:, g * I:(g + 1) * I],
                    in1=w23[:, j * I:(j + 1) * I], op=OP.mult,
                )

            W16v = W16[:].rearrange("p (n k) -> p n k", k=16)
            acc = pool.tile([P, I], F32, tag="acc")

            # group-wise blend: runs as soon as its GB gathers land, while
            # later gathers continue; each group's outputs stream to DRAM
            for grp in range(0, I, GB):
                M = []
                for ab in range(4):
                    a, b = ab >> 1, ab & 1
                    goff = grp * SPANW + a * 512 + b * 64
                    gview = Gbig[:]
                    gview = bass.AP(
                        gview.tensor,
                        gview.offset + goff,
                        [gview.ap[0], [SPANW, GB], [8, 2], [1, 2]],
                    )
                    wview = bass.AP(
                        W16v.tensor,
                        W16v.offset + grp * 16 + ab * 4,
                        [W16v.ap[0], [16, GB], [2, 2], [1, 2]],
                    )
                    m = pool.tile([P, GB * 4], F32, tag=f"M{grp}_{ab}")
                    nc.vector.tensor_tensor(
                        out=m[:].rearrange("p (n c d) -> p n c d", c=2, d=2),
                        in0=gview, in1=wview, op=OP.mult,
                    )
                    M.append(m)
                m01 = pool.tile([P, GB * 4], F32, tag=f"m01_{grp}")
                m23 = pool.tile([P, GB * 4], F32, tag=f"m23_{grp}")
                msum = pool.tile([P, GB * 4], F32, tag=f"msum_{grp}")
                nc.vector.tensor_tensor(out=m01[:], in0=M[0][:], in1=M[1][:], op=OP.add)
                nc.vector.tensor_tensor(out=m23[:], in0=M[2][:], in1=M[3][:], op=OP.add)
                nc.vector.tensor_tensor(out=msum[:], in0=m01[:], in1=m23[:], op=OP.add)
                # reduce (c,d): adjacent pairs twice
                t7 = pool.tile([P, GB * 2], F32, tag=f"t7_{grp}")
                nc.vector.tensor_tensor(
                    out=t7[:], in0=msum[:, 0::2], in1=msum[:, 1::2], op=OP.add
                )
                nc.vector.tensor_tensor(
                    out=acc[:, grp:grp + GB], in0=t7[:, 0::2], in1=t7[:, 1::2],
                    op=OP.add,
                )
                nc.sync.dma_start(
                    out=bass.AP(out_t.tensor, out_t.offset + grp,
                                [out_t.ap[0], [1, GB]]),
                    in_=acc[:, grp:grp + GB],
                )
    nc.compile()
    return nc


_NC = None


def _get_nc():
    global _NC
    if _NC is None:
        _NC = _build()
    return _NC


_ROWF8 = None


def _get_rowf8():
    global _ROWF8
    if _ROWF8 is None:
        n = np.arange(I, dtype=np.float32)[None, :]
        p = np.arange(P, dtype=np.float32)[:, None]
        _ROWF8 = np.ascontiguousarray((n * P + p) * 8.0, dtype=np.float32)
    return _ROWF8


def kernel(coordinates, mesh_pred, _trace=False, _tmpdir=None):
    coordinates = np.asarray(coordinates, dtype=np.float32)
    mesh_pred = np.asarray(mesh_pred, dtype=np.float32)
    assert coordinates.shape == (NCORES * BC, ND)
    assert mesh_pred.shape == (NCORES * BC, VOL)

    in_maps = []
    for cix in range(NCORES):
        sl = slice(cix * BC, (cix + 1) * BC)
        cs = coordinates[sl]
        # device row p*I+n must hold original row n*P+p
        cs_perm = np.ascontiguousarray(
            cs.reshape(I, P, ND).transpose(1, 0, 2).reshape(BC, ND)
        )
        in_maps.append(
            {
                "coordinates": cs_perm,
                "mesh_pred": np.ascontiguousarray(mesh_pred[sl]),
                "rowf8": _get_rowf8(),
            }
        )
    res = bass_utils.run_bass_kernel_spmd(
        _get_nc(),
        in_maps,
        core_ids=list(range(NCORES)),
        trace=_trace,
        tmpdir=_tmpdir,
    )
    outs = []
    for r in res.results:
        o = np.asarray(r["out"]).reshape(P, I)  # [p, n]
        outs.append(o.transpose(1, 0).reshape(-1))  # back to b = n*P+p
    out = np.concatenate(outs)
    if _trace:
        return out, res
    return out


# revision 23
# speedup vs baseline: 1.0041x; 1.0041x over previous
"""4D multilinear interpolation (8x8x8x8 lattice) on 8 Trainium2 cores.

For each row b: scale coordinates[b] (4 values in [0,1)) to the 7-cell
lattice, find the containing cell, gather the 16 corner values from
mesh_pred[b] (4096 values), and blend with multilinear weights.

HW constraint (measured): indirect DMA gather consumes ONE index per
partition and streams the dest free-width contiguously from it (multi-
index offset APs abort at runtime, with or without bounds_check).  So
rows are laid out b = n*128 + p (host pre-permutes coordinates into
(p,n) order; output is permuted back) and each of the 32 gathers
fetches, per partition, the 586-float span that covers all 16 cell
corners of one row.  The gather pitch is ~1.4us/instruction
(SWDGE-paced), so the kernel is structured to hide everything else
under it: the index chain runs in two stages (first 4 row-tiles, then
the rest) so gather 0 issues early, the weight build and group-wise
blends overlap later gathers, and outputs stream out per group.
"""

import numpy as np

import concourse.bass as bass
import concourse.bacc as bacc
import concourse.mybir as mybir
from concourse import bass_utils
from concourse.tile import TileContext

P = 128          # partitions
I = 32           # row-tiles (gathers) per core
GB = 8           # row-tiles per blend group
BC = P * I       # 4096 rows per core
VOL = 4096       # 8^4 lattice values per row
ND = 4
NCORES = 8
MESH = 8
SPANW = 640      # padded per-row gather width (586 used)
SPAN = 586       # 585 max corner offset + 1
F32 = mybir.dt.float32
I32 = mybir.dt.int32
OP = mybir.AluOpType


def _build():
    nc = bacc.Bacc("TRN2", target_bir_lowering=False, debug=False,
                   dynamic_dma_scratch_size=65536, enable_partition_id=False)
    # coordinates arrive host-permuted: device row p*I+n = original row n*P+p
    coords = nc.dram_tensor("coordinates", [BC, ND], F32, kind="ExternalInput")
    mesh = nc.dram_tensor("mesh_pred", [BC, VOL], F32, kind="ExternalInput")
    # host-precomputed (n*P+p)*8 as f32, laid out [p, n] (exact below 2^24)
    rowf8 = nc.dram_tensor("rowf8", [P, I], F32, kind="ExternalInput")
    out = nc.dram_tensor("out", [BC], F32, kind="ExternalOutput")

    mesh_2d = mesh[:]
    coords_t = coords[:].rearrange("(p n) d -> p (n d)", p=P)
    out_t = out[:].rearrange("(p n) -> p n", p=P)  # host permutes back

    with TileContext(nc) as tc:
        with tc.tile_pool(name="pool", bufs=1) as pool:
            ct = pool.tile([P, I * ND], F32, tag="ct")
            nc.sync.dma_start(out=ct[:], in_=coords_t)
            rbase = pool.tile([P, I], F32, tag="rbase")
            nc.scalar.dma_start(out=rbase[:], in_=rowf8[:])

            c = pool.tile([P, I * ND], F32, tag="c")
            frac = pool.tile([P, I * ND], F32, tag="frac")
            cif = pool.tile([P, I * ND], F32, tag="cif")
            ci_r = pool.tile([P, I * ND], I32, tag="ci_r")
            ci_f = pool.tile([P, I * ND], F32, tag="ci_f")
            gtt = pool.tile([P, I * ND], F32, tag="gtt")
            t0 = pool.tile([P, I], F32, tag="t0")
            t1 = pool.tile([P, I], F32, tag="t1")
            t2 = pool.tile([P, I], F32, tag="t2")
            t3 = pool.tile([P, I], F32, tag="t3")
            t4 = pool.tile([P, I], F32, tag="t4")
            t5 = pool.tile([P, I], F32, tag="t5")
            t6 = pool.tile([P, I], F32, tag="t6")
            idx = pool.tile([P, I], I32, tag="idx")

            # idx = (((row*8 + d0)*8 + d1)*8 + d2)*8 + d3, all exact in f32.
            # single full-width stage: a split early stage lets the scheduler
            # interleave the wide stage into the serial chain, which delays
            # the first gather instead of advancing it (measured).
            for lo, hi in ((0, I),):
                cs = slice(lo * ND, hi * ND)
                ns = slice(lo, hi)
                nc.vector.tensor_scalar_mul(c[:, cs], ct[:, cs], float(MESH - 1))
                # floor(c) via round-trip cast + fixup (works for any cast
                # rounding mode: the cast lands on floor or floor+1, is_gt
                # subtracts the overshoot); frac = c - floor(c)
                nc.vector.tensor_copy(out=ci_r[:, cs], in_=c[:, cs])
                nc.vector.tensor_copy(out=ci_f[:, cs], in_=ci_r[:, cs])
                nc.vector.tensor_tensor(
                    out=gtt[:, cs], in0=ci_f[:, cs], in1=c[:, cs], op=OP.is_gt
                )
                nc.vector.tensor_tensor(
                    out=cif[:, cs], in0=ci_f[:, cs], in1=gtt[:, cs], op=OP.subtract
                )
                d = [cif[:, lo * ND + k::ND] for k in range(ND)]
                # slice columns [lo:hi) of the strided view
                d = [bass.AP(v.tensor, v.offset, [v.ap[0], [ND, hi - lo]])
                     for v in d]
                nc.vector.tensor_tensor(
                    out=t0[:, ns], in0=rbase[:, ns], in1=d[0], op=OP.add)
                nc.vector.tensor_scalar_mul(t1[:, ns], t0[:, ns], 8.0)
                nc.vector.tensor_tensor(
                    out=t2[:, ns], in0=t1[:, ns], in1=d[1], op=OP.add)
                nc.vector.tensor_scalar_mul(t3[:, ns], t2[:, ns], 8.0)
                nc.vector.tensor_tensor(
                    out=t4[:, ns], in0=t3[:, ns], in1=d[2], op=OP.add)
                nc.vector.tensor_scalar_mul(t5[:, ns], t4[:, ns], 8.0)
                nc.vector.tensor_tensor(
                    out=t6[:, ns], in0=t5[:, ns], in1=d[3], op=OP.add)
                nc.vector.tensor_copy(out=idx[:, ns], in_=t6[:, ns])

            Gbig = pool.tile([P, I * SPANW], F32, tag="Gbig")
            for n in range(0, I):
                nc.gpsimd.indirect_dma_start(
                    out=Gbig[:, n * SPANW:n * SPANW + SPAN],
                    out_offset=None,
                    in_=mesh_2d,
                    in_offset=bass.IndirectOffsetOnAxis(
                        ap=idx[:, n:n + 1], axis=1),
                    element_offset=0,
                )

            # weights (off the gather critical path): frac = c - floor(c);
            # om=1-frac; w01[(g,n)], w23[(j,n)]; W16[(n,k)] k=(a,b,c,d)
            nc.vector.tensor_tensor(
                out=frac[:], in0=c[:], in1=cif[:], op=OP.subtract
            )
            om = pool.tile([P, I * ND], F32, tag="om")
            nc.vector.tensor_scalar(
                out=om[:], in0=frac[:], scalar1=-1.0, scalar2=1.0,
                op0=OP.mult, op1=OP.add,
            )
            w01 = pool.tile([P, 4 * I], F32, tag="w01")
            w23 = pool.tile([P, 4 * I], F32, tag="w23")
            pairs = ((0, 0), (0, 1), (1, 0), (1, 1))
            for g, (a, b) in enumerate(pairs):
                nc.vector.tensor_tensor(
                    out=w23[:, g * I:(g + 1) * I],
                    in0=(frac if a else om)[:, 2::ND],
                    in1=(frac if b else om)[:, 3::ND], op=OP.mult,
                )
            for g, (a, b) in enumerate(pairs):
                nc.vector.tensor_tensor(
                    out=w01[:, g * I:(g + 1) * I],
                    in0=(frac if a else om)[:, 0::ND],
                    in1=(frac if b else om)[:, 1::ND], op=OP.mult,
                )
            W16 = pool.tile([P, I * 16], F32, tag="W16")  # layout (n, k) k fastest
            for k in range(16):
                g, j = k >> 2, k & 3
                nc.vector.tensor_tensor(
                    out=W16[:, k::16],
                    in0=w01[:, g * I:(g + 1) * I],
                    in1=w23[:, j * I:(j + 1) * I], op=OP.mult,
                )

            W16v = W16[:].rearrange("p (n k) -> p n k", k=16)
            acc = pool.tile([P, I], F32, tag="acc")

            # group-wise blend: runs as soon as its gathers land, while
            # later gathers continue; each group's outputs stream to DRAM.
            # small final groups shorten the post-last-gather tail.
            groups = ((0, 12), (12, 12), (24, 4), (28, 4))
            for grp, gb in groups:
                M = []
                for ab in range(4):
                    a, b = ab >> 1, ab & 1
                    goff = grp * SPANW + a * 512 + b * 64
                    gview = Gbig[:]
                    gview = bass.AP(
                        gview.tensor,
                        gview.offset + goff,
                        [gview.ap[0], [SPANW, gb], [8, 2], [1, 2]],
                    )
                    wview = bass.AP(
                        W16v.tensor,
                        W16v.offset + grp * 16 + ab * 4,
                        [W16v.ap[0], [16, gb], [2, 2], [1, 2]],
                    )
                    m = pool.tile([P, gb * 4], F32, tag=f"M{grp}_{ab}")
                    nc.vector.tensor_tensor(
                        out=m[:].rearrange("p (n c d) -> p n c d", c=2, d=2),
                        in0=gview, in1=wview, op=OP.mult,
                    )
                    M.append(m)
                m01 = pool.tile([P, gb * 4], F32, tag=f"m01_{grp}")
                m23 = pool.tile([P, gb * 4], F32, tag=f"m23_{grp}")
                msum = pool.tile([P, gb * 4], F32, tag=f"msum_{grp}")
                nc.vector.tensor_tensor(out=m01[:], in0=M[0][:], in1=M[1][:], op=OP.add)
                nc.vector.tensor_tensor(out=m23[:], in0=M[2][:], in1=M[3][:], op=OP.add)
                nc.vector.tensor_tensor(out=msum[:], in0=m01[:], in1=m23[:], op=OP.add)
                # reduce (c,d): adjacent pairs twice
                t7 = pool.tile([P, gb * 2], F32, tag=f"t7_{grp}")
                nc.vector.tensor_tensor(
                    out=t7[:], in0=msum[:, 0::2], in1=msum[:, 1::2], op=OP.add
                )
                nc.vector.tensor_tensor(
                    out=acc[:, grp:grp + gb], in0=t7[:, 0::2], in1=t7[:, 1::2],
                    op=OP.add,
                )
                nc.sync.dma_start(
                    out=bass.AP(out_t.tensor, out_t.offset + grp,
                                [out_t.ap[0], [1, gb]]),
                    in_=acc[:, grp:grp + gb],
                )
    nc.compile()
    return nc


_NC = None


def _get_nc():
    global _NC
    if _NC is None:
        _NC = _build()
    return _NC


_ROWF8 = None


def _get_rowf8():
    global _ROWF8
    if _ROWF8 is None:
        n = np.arange(I, dtype=np.float32)[None, :]
        p = np.arange(P, dtype=np.float32)[:, None]
        _ROWF8 = np.ascontiguousarray((n * P + p) * 8.0, dtype=np.float32)
    return _ROWF8


def kernel(coordinates, mesh_pred, _trace=False, _tmpdir=None):
    coordinates = np.asarray(coordinates, dtype=np.float32)
    mesh_pred = np.asarray(mesh_pred, dtype=np.float32)
    assert coordinates.shape == (NCORES * BC, ND)
    assert mesh_pred.shape == (NCORES * BC, VOL)

    in_maps = []
    for cix in range(NCORES):
        sl = slice(cix * BC, (cix + 1) * BC)
        cs = coordinates[sl]
        # device row p*I+n must hold original row n*P+p
        cs_perm = np.ascontiguousarray(
            cs.reshape(I, P, ND).transpose(1, 0, 2).reshape(BC, ND)
        )
        in_maps.append(
            {
                "coordinates": cs_perm,
                "mesh_pred": np.ascontiguousarray(mesh_pred[sl]),
                "rowf8": _get_rowf8(),
            }
        )
    res = bass_utils.run_bass_kernel_spmd(
        _get_nc(),
        in_maps,
        core_ids=list(range(NCORES)),
        trace=_trace,
        tmpdir=_tmpdir,
    )
    outs = []
    for r in res.results:
        o = np.asarray(r["out"]).reshape(P, I)  # [p, n]
        outs.append(o.transpose(1, 0).reshape(-1))  # back to b = n*P+p
    out = np.concatenate(outs)
    if _trace:
        return out, res
    return out


# revision 24
# speedup vs baseline: 1.0119x; 1.0077x over previous
"""4D multilinear interpolation (8x8x8x8 lattice) on 8 Trainium2 cores.

For each row b: scale coordinates[b] (4 values in [0,1)) to the 7-cell
lattice, find the containing cell, gather the 16 corner values from
mesh_pred[b] (4096 values), and blend with multilinear weights.

HW constraint (measured): indirect DMA gather consumes ONE index per
partition and streams the dest free-width contiguously from it (multi-
index offset APs abort at runtime, with or without bounds_check).  So
rows are laid out b = n*128 + p (host pre-permutes coordinates into
(p,n) order; output is permuted back) and each of the 32 gathers
fetches, per partition, the 586-float span that covers all 16 cell
corners of one row.  The gather pitch is ~1.4us/instruction
(SWDGE-paced), so the kernel is structured to hide everything else
under it: the index chain runs in two stages (first 4 row-tiles, then
the rest) so gather 0 issues early, the weight build and group-wise
blends overlap later gathers, and outputs stream out per group.
"""

import numpy as np

import concourse.bass as bass
import concourse.bacc as bacc
import concourse.mybir as mybir
from concourse import bass_utils
from concourse.tile import TileContext

P = 128          # partitions
I = 32           # row-tiles (gathers) per core
GB = 8           # row-tiles per blend group
BC = P * I       # 4096 rows per core
VOL = 4096       # 8^4 lattice values per row
ND = 4
NCORES = 8
MESH = 8
SPANW = 640      # padded per-row gather width (586 used)
SPAN = 586       # 585 max corner offset + 1
F32 = mybir.dt.float32
I32 = mybir.dt.int32
OP = mybir.AluOpType


def _build():
    nc = bacc.Bacc("TRN2", target_bir_lowering=False, debug=False,
                   dynamic_dma_scratch_size=65536, enable_partition_id=False)
    # coordinates arrive host-permuted: device row p*I+n = original row n*P+p
    coords = nc.dram_tensor("coordinates", [BC, ND], F32, kind="ExternalInput")
    mesh = nc.dram_tensor("mesh_pred", [BC, VOL], F32, kind="ExternalInput")
    # host-precomputed (n*P+p)*8 as f32, laid out [p, n] (exact below 2^24)
    rowf8 = nc.dram_tensor("rowf8", [P, I], F32, kind="ExternalInput")
    out = nc.dram_tensor("out", [BC], F32, kind="ExternalOutput")

    mesh_2d = mesh[:]
    coords_t = coords[:].rearrange("(p n) d -> p (n d)", p=P)
    out_t = out[:].rearrange("(p n) -> p n", p=P)  # host permutes back

    with TileContext(nc) as tc:
        with tc.tile_pool(name="pool", bufs=1) as pool:
            ct = pool.tile([P, I * ND], F32, tag="ct")
            nc.sync.dma_start(out=ct[:], in_=coords_t)
            rbase = pool.tile([P, I], F32, tag="rbase")
            nc.scalar.dma_start(out=rbase[:], in_=rowf8[:])

            c = pool.tile([P, I * ND], F32, tag="c")
            frac = pool.tile([P, I * ND], F32, tag="frac")
            cif = pool.tile([P, I * ND], F32, tag="cif")
            ci_r = pool.tile([P, I * ND], I32, tag="ci_r")
            ci_f = pool.tile([P, I * ND], F32, tag="ci_f")
            gtt = pool.tile([P, I * ND], F32, tag="gtt")
            t0 = pool.tile([P, I], F32, tag="t0")
            t1 = pool.tile([P, I], F32, tag="t1")
            t2 = pool.tile([P, I], F32, tag="t2")
            t3 = pool.tile([P, I], F32, tag="t3")
            t4 = pool.tile([P, I], F32, tag="t4")
            t5 = pool.tile([P, I], F32, tag="t5")
            t6 = pool.tile([P, I], F32, tag="t6")
            idx = pool.tile([P, I], I32, tag="idx")

            # idx = (((row*8 + d0)*8 + d1)*8 + d2)*8 + d3, all exact in f32.
            # single full-width stage: a split early stage lets the scheduler
            # interleave the wide stage into the serial chain, which delays
            # the first gather instead of advancing it (measured).
            for lo, hi in ((0, I),):
                cs = slice(lo * ND, hi * ND)
                ns = slice(lo, hi)
                nc.vector.tensor_scalar_mul(c[:, cs], ct[:, cs], float(MESH - 1))
                # floor(c) via round-trip cast + fixup (works for any cast
                # rounding mode: the cast lands on floor or floor+1, is_gt
                # subtracts the overshoot); frac = c - floor(c)
                nc.vector.tensor_copy(out=ci_r[:, cs], in_=c[:, cs])
                nc.vector.tensor_copy(out=ci_f[:, cs], in_=ci_r[:, cs])
                nc.vector.tensor_tensor(
                    out=gtt[:, cs], in0=ci_f[:, cs], in1=c[:, cs], op=OP.is_gt
                )
                nc.vector.tensor_tensor(
                    out=cif[:, cs], in0=ci_f[:, cs], in1=gtt[:, cs], op=OP.subtract
                )
                d = [cif[:, lo * ND + k::ND] for k in range(ND)]
                # slice columns [lo:hi) of the strided view
                d = [bass.AP(v.tensor, v.offset, [v.ap[0], [ND, hi - lo]])
                     for v in d]
                nc.vector.tensor_tensor(
                    out=t0[:, ns], in0=rbase[:, ns], in1=d[0], op=OP.add)
                nc.vector.tensor_scalar_mul(t1[:, ns], t0[:, ns], 8.0)
                nc.vector.tensor_tensor(
                    out=t2[:, ns], in0=t1[:, ns], in1=d[1], op=OP.add)
                nc.vector.tensor_scalar_mul(t3[:, ns], t2[:, ns], 8.0)
                nc.vector.tensor_tensor(
                    out=t4[:, ns], in0=t3[:, ns], in1=d[2], op=OP.add)
                nc.vector.tensor_scalar_mul(t5[:, ns], t4[:, ns], 8.0)
                nc.vector.tensor_tensor(
                    out=t6[:, ns], in0=t5[:, ns], in1=d[3], op=OP.add)
                nc.vector.tensor_copy(out=idx[:, ns], in_=t6[:, ns])

            Gbig = pool.tile([P, I * SPANW], F32, tag="Gbig")
            for n in range(0, I):
                nc.gpsimd.indirect_dma_start(
                    out=Gbig[:, n * SPANW:n * SPANW + SPAN],
                    out_offset=None,
                    in_=mesh_2d,
                    in_offset=bass.IndirectOffsetOnAxis(
                        ap=idx[:, n:n + 1], axis=1),
                    element_offset=0,
                )

            # weights (off the gather critical path): frac = c - floor(c);
            # om=1-frac; w01[(g,n)], w23[(j,n)]; W16[(n,k)] k=(a,b,c,d)
            nc.vector.tensor_tensor(
                out=frac[:], in0=c[:], in1=cif[:], op=OP.subtract
            )
            om = pool.tile([P, I * ND], F32, tag="om")
            nc.vector.tensor_scalar(
                out=om[:], in0=frac[:], scalar1=-1.0, scalar2=1.0,
                op0=OP.mult, op1=OP.add,
            )
            w01 = pool.tile([P, 4 * I], F32, tag="w01")
            w23 = pool.tile([P, 4 * I], F32, tag="w23")
            pairs = ((0, 0), (0, 1), (1, 0), (1, 1))
            for g, (a, b) in enumerate(pairs):
                nc.vector.tensor_tensor(
                    out=w23[:, g * I:(g + 1) * I],
                    in0=(frac if a else om)[:, 2::ND],
                    in1=(frac if b else om)[:, 3::ND], op=OP.mult,
                )
            for g, (a, b) in enumerate(pairs):
                nc.vector.tensor_tensor(
                    out=w01[:, g * I:(g + 1) * I],
                    in0=(frac if a else om)[:, 0::ND],
                    in1=(frac if b else om)[:, 1::ND], op=OP.mult,
                )
            W16 = pool.tile([P, I * 16], F32, tag="W16")  # layout (n, k) k fastest
            for k in range(16):
                g, j = k >> 2, k & 3
                nc.vector.tensor_tensor(
                    out=W16[:, k::16],
                    in0=w01[:, g * I:(g + 1) * I],
                    in1=w23[:, j * I:(j + 1) * I], op=OP.mult,
                )

            W16v = W16[:].rearrange("p (n k) -> p n k", k=16)
            acc = pool.tile([P, I], F32, tag="acc")

            # group-wise blend: runs as soon as its gathers land, while
            # later gathers continue; each group's outputs stream to DRAM.
            # small final groups shorten the post-last-gather tail.
            groups = ((0, 12), (12, 12), (24, 4), (28, 4))
            for grp, gb in groups:
                # products land in one (n, ab, cd)-tile, then a single
                # grouped tensor_reduce over the 16 corners per row
                M4 = pool.tile([P, gb * 16], F32, tag=f"M4_{grp}")
                m4f = M4[:]
                for ab in range(4):
                    a, b = ab >> 1, ab & 1
                    goff = grp * SPANW + a * 512 + b * 64
                    gview = Gbig[:]
                    gview = bass.AP(
                        gview.tensor,
                        gview.offset + goff,
                        [gview.ap[0], [SPANW, gb], [8, 2], [1, 2]],
                    )
                    wview = bass.AP(
                        W16v.tensor,
                        W16v.offset + grp * 16 + ab * 4,
                        [W16v.ap[0], [16, gb], [2, 2], [1, 2]],
                    )
                    mview = bass.AP(
                        m4f.tensor,
                        m4f.offset + ab * 4,
                        [m4f.ap[0], [16, gb], [2, 2], [1, 2]],
                    )
                    nc.vector.tensor_tensor(
                        out=mview, in0=gview, in1=wview, op=OP.mult,
                    )
                nc.vector.tensor_reduce(
                    out=acc[:, grp:grp + gb],
                    in_=m4f.rearrange("p (n k) -> p n k", k=16),
                    axis=mybir.AxisListType.X,
                    op=OP.add,
                )
                nc.sync.dma_start(
                    out=bass.AP(out_t.tensor, out_t.offset + grp,
                                [out_t.ap[0], [1, gb]]),
                    in_=acc[:, grp:grp + gb],
                )
    nc.compile()
    return nc


_NC = None


def _get_nc():
    global _NC
    if _NC is None:
        _NC = _build()
    return _NC


_ROWF8 = None


def _get_rowf8():
    global _ROWF8
    if _ROWF8 is None:
        n = np.arange(I, dtype=np.float32)[None, :]
        p = np.arange(P, dtype=np.float32)[:, None]
        _ROWF8 = np.ascontiguousarray((n * P + p) * 8.0, dtype=np.float32)
    return _ROWF8


def kernel(coordinates, mesh_pred, _trace=False, _tmpdir=None):
    coordinates = np.asarray(coordinates, dtype=np.float32)
    mesh_pred = np.asarray(mesh_pred, dtype=np.float32)
    assert coordinates.shape == (NCORES * BC, ND)
    assert mesh_pred.shape == (NCORES * BC, VOL)

    in_maps = []
    for cix in range(NCORES):
        sl = slice(cix * BC, (cix + 1) * BC)
        cs = coordinates[sl]
        # device row p*I+n must hold original row n*P+p
        cs_perm = np.ascontiguousarray(
            cs.reshape(I, P, ND).transpose(1, 0, 2).reshape(BC, ND)
        )
        in_maps.append(
            {
                "coordinates": cs_perm,
                "mesh_pred": np.ascontiguousarray(mesh_pred[sl]),
                "rowf8": _get_rowf8(),
            }
        )
    res = bass_utils.run_bass_kernel_spmd(
        _get_nc(),
        in_maps,
        core_ids=list(range(NCORES)),
        trace=_trace,
        tmpdir=_tmpdir,
    )
    outs = []
    for r in res.results:
        o = np.asarray(r["out"]).reshape(P, I)  # [p, n]
        outs.append(o.transpose(1, 0).reshape(-1))  # back to b = n*P+p
    out = np.concatenate(outs)
    if _trace:
        return out, res
    return out
